# revision 19
# baseline (speedup 1.0000x reference)
"""Trainium2 Bass kernel: per-pixel 5x5 kernel application (KPN-style).

    out[b,c,y,x] = sum_{i,j} softmax(kernels[b,:,y,x])[i*5+j]
                   * zpad(data)[b,c,y+i,x+j]          (i,j in 0..4, r=2)

Sharding (8 NeuronCores, pure data parallel, no collectives):
    core = (b, H-half): 4 batches x 2 row-bands of 360 rows.
    Halo rows come from overlapping host-side slices of the full input.

The per-core HBM pipe sustains only ~92 GB/s regardless of DMA engine
spreading, so runtime is dominated by bytes moved. Traffic reduction:
    - kernel tensor ships as int8 with a per-(row, di-group) affine
      dequant (scale/bias), applied FOR FREE inside the ACT exp
      (exp(scale*k + bias)); 23MB -> 11.5MB. Measured rel-l2 vs f32
      reference: 8.1e-3 (gate 2e-2).
    - data ships bf16; output stores bf16 (upcast on host).
    Total ~17.2MB/core -> ~187us DMA floor at 92GB/s.

Compute (overlapped under the DMA stream):
    - kernel taps arrive as 5 di-group DMAs per row-tile (rows shifted
      by -di, 5 dj taps DRAM-contiguous: 124 x 6.4KB descriptors) on the
      software DGE queue, which nothing else blocks.
    - ACT: exp per (x-chunk, di-group) with int8 in, bf16 out, dequant
      scale/bias as per-partition operands.
    - DVE: tap products q = e * d in bf16 2x; one instruction covers the
      even (or odd) dj taps of a group via overlapping-window APs; two
      parity copies of the data keep operands 4B-aligned.
    - PE: stationary shift matrix S_di[k,m] = [k == m+di] undoes the
      load shift; accumulates 25 q planes per channel + 25 exp planes
      (softmax denominator) into PSUM.
    - DVE: out_c = PSUM_c * reciprocal(PSUM_sum) -> bf16.

Queue assignment (stall avoidance): SWDGE carries only kq loads; the
scalar ring carries data/scale loads (issued before that engine's exps);
the sync ring carries stores (the sync engine runs nothing else, so
store semaphore waits never block loads or compute).

kernel(**inputs) takes the FULL inputs and returns the FULL output.
"""

import numpy as np
import ml_dtypes

B, C, H, W, KW = 4, 3, 720, 1280, 5
NCORES = 8
HS = H // 2            # 360 output rows per shard
RT = 120               # output rows per row-tile
NRT = HS // RT         # 3 row-tiles
HALO = 2
DP = RT + 2 * HALO     # 124 partitions (data space)
WP = 1288              # padded data width: 2 left + 1280 + 6 right
KROWPAD = 4            # zero rows around each kernel shard (top+bottom)
KH = HS + 2 * KROWPAD  # 368
XCH = [(0, 512), (512, 512), (1024, 256)]

RECIP_ACT = True       # reciprocal via ACT ln/exp instead of DVE Newton

_CACHE = {}


def _build_program():
    import concourse.bacc as bacc
    import concourse.mybir as mybir
    from concourse.bass import AP
    from concourse import tile

    f32 = mybir.dt.float32
    bf16 = mybir.dt.bfloat16
    i8 = mybir.dt.int8

    nc = bacc.Bacc(
        "TRN2",
        target_bir_lowering=False,
        debug=False,
        enable_asserts=False,
        num_devices=NCORES,
    )
    d_data = nc.dram_tensor("data", [HS + 2 * HALO, C, WP], bf16, kind="ExternalInput")
    d_kq = nc.dram_tensor("kq", [KH, KW * KW, W], i8, kind="ExternalInput")
    d_scb = nc.dram_tensor("scb", [KH, KW, 2], f32, kind="ExternalInput")
    d_out = nc.dram_tensor("out", [HS, C, W], bf16, kind="ExternalOutput")

    # Shift matrices S_di[k, m] = 1 iff k == m + di  (k: 124 data rows,
    # m: 120 out rows). Baked into the NEFF as a Const tensor.
    s_np = np.zeros((KW, DP, RT), dtype=ml_dtypes.bfloat16)
    for di in range(KW):
        for m in range(RT):
            s_np[di, m + di, m] = 1.0
    d_s = nc.inline_tensor(np.ascontiguousarray(s_np), "smat")

    KROW = KW * KW * W  # element stride between rows of d_kq

    with tile.TileContext(nc) as tc:
        with tc.tile_pool(name="const", bufs=1) as cpool, \
             tc.tile_pool(name="dbf", bufs=2) as dbfpool, \
             tc.tile_pool(name="kq", bufs=10) as kqpool, \
             tc.tile_pool(name="scb", bufs=3) as scbpool, \
             tc.tile_pool(name="ech", bufs=11) as epool, \
             tc.tile_pool(name="qt", bufs=2) as qpool, \
             tc.tile_pool(name="fin", bufs=2) as fpool, \
             tc.tile_pool(name="ps", bufs=2, space="PSUM") as ppool:

            s_sb = cpool.tile([DP, KW, RT], bf16)
            nc.sync.dma_start(out=s_sb[:], in_=d_s.ap().transpose([1, 0, 2]))

            # Chunk finals (recip + normalize) are deferred until the next
            # chunk's products have been issued, so the PE-drain wait never
            # head-of-line blocks the ACT exp stream or the DVE product
            # stream.
            pending_final = []

            def flush_final():
                while pending_final:
                    pending_final.pop()()

            for rt in range(NRT):
                y0 = rt * RT

                # kernel di-group tiles, rows shifted by -di:
                # kq[di][p, dj, x] = kq8[y0 + p - di, 5*di + dj, x];
                # one DMA per group, 124 x 6.4KB descriptors. rt0 rides the
                # sync HWDGE ring (prompt completion semaphores at startup);
                # later row-tiles stream on the SWDGE queue, whose laggy
                # semaphores are hidden by the kq pool lookahead.
                kq_eng = nc.sync if rt == 0 else nc.gpsimd
                kqs = []
                for di in range(KW):
                    kq = kqpool.tile([DP, KW, W], i8, tag="kq")
                    off = (KROWPAD + y0 - di) * KROW + di * KW * W
                    kq_eng.dma_start(
                        out=kq[:],
                        in_=AP(d_kq, off, [[KROW, DP], [1, KW * W]]),
                    )
                    kqs.append(kq)
                # dequant scale/bias for all 5 groups, row shifts baked into
                # the host layout: scball[p, di] = (s, b) of kernel row
                # y0 + p - di. One 124 x 40B DMA per row-tile.
                scball = scbpool.tile([DP, KW, 2], f32, tag="scb")
                nc.scalar.dma_start(
                    out=scball[:],
                    in_=AP(d_scb, (KROWPAD + y0) * KW * 2, [[KW * 2, DP], [1, KW * 2]]),
                )
                # data rows y0-2 .. y0+121 (host-padded), bf16
                dbf0 = dbfpool.tile([DP, C, WP], bf16, tag="dbf0")
                dbf1 = dbfpool.tile([DP, C, WP], bf16, tag="dbf1")
                nc.scalar.dma_start(
                    out=dbf0[:], in_=d_data.ap()[y0:y0 + DP],
                )
                # dbf1 = dbf0 shifted one element left (odd-dj 4B alignment);
                # on gpsimd to keep the DVE stream pure products.
                f0 = dbf0[:].rearrange("p c w -> p (c w)")
                f1 = dbf1[:].rearrange("p c w -> p (c w)")
                nc.gpsimd.tensor_copy(f1[:, 0:C * WP - 1], f0[:, 1:C * WP])

                rs = fpool.tile([RT, W], f32, tag="rs", bufs=1)
                ost = fpool.tile([RT, C, W], bf16, tag="ost")

                dbf0_ap = dbf0[:]
                dbf1_ap = dbf1[:]
                dp_stride = dbf0_ap.ap[0][0]

                for (xc, xcw) in XCH:
                    # PSUM banks: 0..2 = channel accumulators, 3 = sumexp
                    pacc = ppool.tile([RT, 4, 512], f32, tag="pacc")

                    ech = []
                    for di in range(KW):
                        e = epool.tile([DP, KW, 512], bf16, tag="ech")
                        nc.scalar.activation(
                            e[:, :, 0:xcw],
                            kqs[di][:, :, xc:xc + xcw],
                            mybir.ActivationFunctionType.Exp,
                            bias=scball[:, di, 1:2],
                            scale=scball[:, di, 0:1],
                        )
                        ech.append(e)

                    # sumexp first: a continuous 25-matmul run (only needs
                    # the exp tiles) that lets the PE clock ramp up
                    for di in range(KW):
                        for dj in range(KW):
                            nc.tensor.matmul(
                                out=pacc[:, 3, 0:xcw],
                                lhsT=s_sb[:, di, :],
                                rhs=ech[di][:, dj, 0:xcw],
                                start=di == 0 and dj == 0,
                                stop=di == KW - 1 and dj == KW - 1,
                            )

                    for di in range(KW):
                        e = ech[di]
                        lhs = s_sb[:, di, :]
                        first = di == 0
                        last = di == KW - 1
                        # tap products q[p, dj, c, x] = e[p, dj, x] *
                        # d[p, c, x + dj]; one DVE instruction per parity
                        # (overlapping dj windows, stride 2, on dbf0/dbf1).
                        qt = qpool.tile([DP, KW, C, 512], bf16, tag="qt")
                        e_ev = (
                            e[:, 0:KW:2, 0:xcw]
                            .unsqueeze(2).broadcast_to([DP, 3, C, xcw])
                        )
                        d_ev = AP(
                            dbf0_ap.tensor,
                            dbf0_ap.offset + xc,
                            [[dp_stride, DP], [2, 3], [WP, C], [1, xcw]],
                        )
                        nc.vector.tensor_tensor(
                            qt[:, 0:KW:2, :, 0:xcw], e_ev, d_ev,
                            mybir.AluOpType.mult,
                        )
                        e_od = (
                            e[:, 1:KW:2, 0:xcw]
                            .unsqueeze(2).broadcast_to([DP, 2, C, xcw])
                        )
                        d_od = AP(
                            dbf1_ap.tensor,
                            dbf1_ap.offset + xc,
                            [[dp_stride, DP], [2, 2], [WP, C], [1, xcw]],
                        )
                        nc.vector.tensor_tensor(
                            qt[:, 1:KW:2, :, 0:xcw], e_od, d_od,
                            mybir.AluOpType.mult,
                        )

                        for dj in range(KW):
                            for c in range(C):
                                nc.tensor.matmul(
                                    out=pacc[:, c, 0:xcw],
                                    lhsT=lhs,
                                    rhs=qt[:, dj, c, 0:xcw],
                                    start=first and dj == 0,
                                    stop=last and dj == KW - 1,
                                )

                    def make_final(pacc=pacc, rs=rs, ost=ost, xc=xc, xcw=xcw):
                        def fin():
                            if RECIP_ACT:
                                lnt = fpool.tile([RT, 512], f32, tag="lnt", bufs=1)
                                nc.scalar.activation(
                                    lnt[:, 0:xcw], pacc[:, 3, 0:xcw],
                                    mybir.ActivationFunctionType.Ln,
                                )
                                nc.scalar.activation(
                                    rs[:, xc:xc + xcw], lnt[:, 0:xcw],
                                    mybir.ActivationFunctionType.Exp,
                                    scale=-1.0,
                                )
                            else:
                                nc.vector.reciprocal(
                                    rs[:, xc:xc + xcw], pacc[:, 3, 0:xcw]
                                )
                            rsb = (
                                rs[:, xc:xc + xcw]
                                .unsqueeze(1).broadcast_to([RT, C, xcw])
                            )
                            nc.vector.tensor_tensor(
                                ost[:, :, xc:xc + xcw], pacc[:, 0:3, 0:xcw],
                                rsb, mybir.AluOpType.mult,
                            )
                        return fin

                    flush_final()
                    pending_final.append(make_final())

                flush_final()
                nc.sync.dma_start(out=d_out.ap()[y0:y0 + RT], in_=ost[:])

    nc.compile()
    return nc


def get_program():
    if "nc" not in _CACHE:
        _CACHE["nc"] = _build_program()
    return _CACHE["nc"]


def make_shards(data: np.ndarray, kernels: np.ndarray):
    """Full inputs -> per-core input maps (quantized kernels + halo pad)."""
    data = np.asarray(data, dtype=np.float32)
    kernels = np.asarray(kernels, dtype=np.float32)
    # zero-pad data: 2 rows top/bottom, 2 cols left, 6 cols right;
    # row-major layouts: data [row, c, x], kern [row, tap, x]
    dpad = np.zeros((B, H + 2 * HALO, C, WP), dtype=ml_dtypes.bfloat16)
    dpad[:, HALO:HALO + H, :, HALO:HALO + W] = (
        data.transpose(0, 2, 1, 3).astype(ml_dtypes.bfloat16)
    )
    # int8 affine quantization per (b, di-group, row): k ~ s*q + bb
    kg = kernels.reshape(B, KW, KW, H, W)
    mx = kg.max(axis=(2, 4))                        # [B, KW, H]
    mn = kg.min(axis=(2, 4))
    s = np.maximum((mx - mn) / 255.0, 1e-30)
    q = np.clip(
        np.rint((kg - mn[:, :, None, :, None]) / s[:, :, None, :, None]) - 128.0,
        -128, 127,
    ).astype(np.int8)                               # [B, KW, KW, H, W]
    bb = mn + 128.0 * s                             # k ~ s*q + bb
    in_maps = []
    for core in range(NCORES):
        b, hh = divmod(core, 2)
        r0 = hh * HS
        dsh = np.ascontiguousarray(dpad[b, r0:r0 + HS + 2 * HALO])
        kq = np.zeros((KH, KW * KW, W), dtype=np.int8)
        kq[KROWPAD:KROWPAD + HS] = (
            q[b].reshape(KW * KW, H, W)[:, r0:r0 + HS, :].transpose(1, 0, 2)
        )
        # pre-shifted: scb[rho, di] = (s, b) of kernel row rho-KROWPAD-di,
        # so one [124, 5, 2] load per row-tile serves all 5 di groups
        scb = np.zeros((KH, KW, 2), dtype=np.float32)
        for di in range(KW):
            scb[KROWPAD + di:KROWPAD + di + HS, di, 0] = s[b, di, r0:r0 + HS]
            scb[KROWPAD + di:KROWPAD + di + HS, di, 1] = bb[b, di, r0:r0 + HS]
        in_maps.append({"data": dsh, "kq": kq, "scb": scb})
    return in_maps


def assemble(results) -> np.ndarray:
    out = np.empty((B, C, H, W), dtype=np.float32)
    for core in range(NCORES):
        b, hh = divmod(core, 2)
        out[b, :, hh * HS:(hh + 1) * HS, :] = (
            results[core]["out"].astype(np.float32).transpose(1, 0, 2)
        )
    return out


def kernel(data: np.ndarray, kernels: np.ndarray) -> np.ndarray:
    from concourse.bass_utils import run_bass_kernel_spmd

    nc = get_program()
    in_maps = make_shards(data, kernels)
    res = run_bass_kernel_spmd(nc, in_maps, list(range(NCORES)))
    return assemble(res.results)


if __name__ == "__main__":
    get_program()
    print("program built OK")
